# revision 26
# baseline (speedup 1.0000x reference)
"""Fused conv-attention kernel for Trainium2, sharded over 8 NeuronCores.

Reference computation (B=2, H=12, L=T=1024, D=64, FEA=3, DIM=768):
    scores = concat([s0,s1,s2], ch)            # [b, 36, l, t]
    fused  = einsum('bclt,oc->bolt', scores, fuse_w) + fuse_b
    attn   = softmax(fused, axis=-1)
    x      = einsum('bhlt,bhtd->bhld', attn, v)
    y      = merge_heads(x) @ proj_w.T + proj_b  # [b, l, 768]

Sharding: fully data-parallel over (b, l-block): core k handles b=k//4 and
l-rows [256*(k%4), 256*(k%4)+256).  Every op is local; no collectives.

All HBM traffic is bf16 (host casts/packs): scores 19.2MB/core with 2KB
descriptors so the DMA stripes across all 16 engines; v/proj weights flow on
the second HWDGE queue (scalar engine) in parallel with the score stream.

Per-core dataflow (G=10 l-rows per group, KM=120 partitions, LCP=260 padded):
  - conv as block-diag matmul: lhsT_j [120,120] holds fuse_w cols for score
    tensor j replicated block-diagonally over lg; j-outer loop -> 3 stationary
    loads per group, 6 matmuls of N=512 into PSUM [120,1024].
  - exp via ScalarE activation (bias=fuse_b, accum_out=row sums z); max
    subtraction skipped (|fused| <= ~5 so fp32 exp is safe).
  - softmax normalize FUSED into the PE transpose: stationary = et t-chunk,
    moving = diag(1/z) bf16 -> psum tp[t,120] holds normalized attn^T.
  - attnT sbuf layout [t-part, (tc, g, o, lg)] bf16: psum->sbuf copies are
    contiguous; phase-2 matmul rhs reads the strided (g,lg) view (N=260).
  - attn @ V: per (h, tc) matmul accumulating x^T [64, 260] over 8 t-chunks.
  - final proj: out[l,768] = sum_hp xT_hp^T @ pw_hp, bias added by DVE.
"""

import sys

import numpy as np

sys.path.insert(0, "/opt/trn_rl_repo")

B, H, L, T, D = 2, 12, 1024, 1024, 64
DIM = H * D  # 768
NCORES = 8
LC = L * B // NCORES  # 256 l-rows per core
G = 10  # l-rows per conv group
NG = 26  # groups per core (last is padded: 260 > 256)
LCP = NG * G  # 260
KM = 12 * G  # 120: conv matmul K and M
NTC = T // 128  # 8 t-chunks

_CACHE = {}


def _build_nc():
    import concourse.bacc as bacc
    import concourse.mybir as mybir
    import concourse.tile as tile
    from contextlib import ExitStack

    f32 = mybir.dt.float32
    bf16 = mybir.dt.bfloat16

    nc = bacc.Bacc(
        "TRN2", target_bir_lowering=False, debug=False, enable_asserts=False
    )

    i8 = mybir.dt.int8
    sc_in = nc.dram_tensor("sc", [NG, KM, 3 * T], i8, kind="ExternalInput").ap()
    v_in = nc.dram_tensor("vp", [H, 128, NTC * D], bf16, kind="ExternalInput").ap()
    w_in = nc.dram_tensor("wts", [3, KM, KM], bf16, kind="ExternalInput").ap()
    b_in = nc.dram_tensor("b120", [KM, 1], f32, kind="ExternalInput").ap()
    id_in = nc.dram_tensor("identb", [KM, KM], bf16, kind="ExternalInput").ap()
    pw_in = nc.dram_tensor("pwp", [128, 6 * DIM], bf16, kind="ExternalInput").ap()
    pb_in = nc.dram_tensor("pbb", [128, DIM], f32, kind="ExternalInput").ap()
    out_d = nc.dram_tensor("out", [LC, DIM], f32, kind="ExternalOutput").ap()

    with tile.TileContext(nc) as tc, ExitStack() as ctx:
        # ---- persistent SBUF (weights arrive on the scalar HWDGE queue) ----
        singles = ctx.enter_context(tc.tile_pool(name="singles", bufs=1))
        wt = [singles.tile([KM, KM], bf16, tag=f"wt{j}", name=f"wt{j}") for j in range(3)]
        for j in range(3):
            nc.scalar.dma_start(wt[j][:], w_in[j])
        b120 = singles.tile([KM, 1], f32)
        nc.scalar.dma_start(b120[:], b_in)
        identb = singles.tile([KM, KM], bf16)
        nc.scalar.dma_start(identb[:], id_in)
        vsb = singles.tile([128, H * NTC * D], bf16)  # [t%128, h*512 + tc*64 + d]
        for h in range(H):
            nc.scalar.dma_start(vsb[:, h * 512 : (h + 1) * 512], v_in[h])
        pw = singles.tile([128, 6 * DIM], bf16)  # [i%128, hp*768 + o]
        for q in range(3):
            nc.scalar.dma_start(
                pw[:, q * 1536 : (q + 1) * 1536], pw_in[:, q * 1536 : (q + 1) * 1536]
            )
        pb = singles.tile([128, DIM], f32)
        nc.scalar.dma_start(pb[:], pb_in)
        # attn^T accumulator: [t%128, tc*3120 + o*260 + (g*10+lg)]  bf16
        attnT = singles.tile([128, NTC * NG * KM], bf16)
        # x^T for proj: [i%128 part, (i//128)*260 + l]  bf16
        xT = singles.tile([128, 6 * LCP], bf16)

        # ---- phase 1: conv + softmax + normalized transpose, per group ----
        with ExitStack() as p1:
            spool = p1.enter_context(tc.tile_pool(name="scores", bufs=3))
            bpool = p1.enter_context(tc.tile_pool(name="sbf", bufs=3))
            fpsum = p1.enter_context(
                tc.tile_pool(name="fpsum", bufs=2, space="PSUM")
            )
            epool = p1.enter_context(tc.tile_pool(name="exp", bufs=3))
            zpool = p1.enter_context(tc.tile_pool(name="z", bufs=4))
            dpool = p1.enter_context(tc.tile_pool(name="diag", bufs=2))
            tpsum = p1.enter_context(
                tc.tile_pool(name="tpsum", bufs=4, space="PSUM")
            )
            for g in range(NG):
                st8 = spool.tile([KM, 3 * T], i8, tag="st8", name=f"st8_{g}")
                nc.sync.dma_start(st8[:], sc_in[g])
                st = bpool.tile([KM, 3 * T], bf16, tag="st", name=f"st_{g}")
                # int8 -> bf16 upcast (exact; 6/127 scale folded into wts):
                # split across DVE and GpSimd so neither becomes critical
                nc.vector.tensor_copy(st[:, 0 : 2 * T], st8[:, 0 : 2 * T])
                nc.gpsimd.tensor_copy(st[:, 2 * T : 3 * T], st8[:, 2 * T : 3 * T])
                fp = fpsum.tile([KM, T], f32)
                for j in range(3):
                    for th in range(2):
                        nc.tensor.matmul(
                            fp[:, th * 512 : (th + 1) * 512],
                            wt[j][:],
                            st[:, j * T + th * 512 : j * T + (th + 1) * 512],
                            start=(j == 0),
                            stop=(j == 2),
                        )
                et = epool.tile([KM, T], bf16, tag="et")
                zt = zpool.tile([KM, 1], f32, tag="zt")
                nc.scalar.activation(
                    et[:],
                    fp[:],
                    mybir.ActivationFunctionType.Exp,
                    bias=b120[:],
                    accum_out=zt[:],
                )
                zi = zpool.tile([KM, 1], f32, tag="zi")
                nc.vector.reciprocal(zi[:], zt[:])
                dg = dpool.tile([KM, KM], bf16, tag="dg")
                nc.vector.tensor_scalar_mul(dg[:], identb[:], zi[:])
                for half in range(2):
                    tp = tpsum.tile([128, 4 * KM], f32)
                    for q in range(4):
                        tc_i = half * 4 + q
                        nc.tensor.matmul(
                            tp[:, q * KM : (q + 1) * KM],
                            et[:, tc_i * 128 : (tc_i + 1) * 128],
                            dg[:],
                            start=True,
                            stop=True,
                        )
                    dst = (
                        attnT[:]
                        .rearrange("p (tc o l) -> p tc o l", tc=NTC, o=H)[
                            :, half * 4 : (half + 1) * 4, :, g * G : (g + 1) * G
                        ]
                    )
                    nc.vector.tensor_copy(
                        dst, tp[:].rearrange("p (q o lg) -> p q o lg", q=4, o=H)
                    )

        # ---- phase 2: attn @ V  -> x^T ----
        with ExitStack() as p2:
            xpsum = p2.enter_context(
                tc.tile_pool(name="xpsum", bufs=4, space="PSUM")
            )
            for h in range(H):
                xp = xpsum.tile([D, LCP], f32)
                for tc_i in range(NTC):
                    nc.tensor.matmul(
                        xp[:],
                        vsb[:, h * 512 + tc_i * D : h * 512 + (tc_i + 1) * D],
                        attnT[
                            :,
                            tc_i * H * LCP + h * LCP : tc_i * H * LCP + (h + 1) * LCP,
                        ],
                        start=(tc_i == 0),
                        stop=(tc_i == NTC - 1),
                    )
                po = (h % 2) * D
                ko = (h // 2) * LCP
                nc.vector.tensor_copy(xT[po : po + D, ko : ko + LCP], xp[:])

            # ---- phase 3: proj -> out ----
            ppsum = p2.enter_context(
                tc.tile_pool(name="ppsum", bufs=2, space="PSUM")
            )
            ypool = p2.enter_context(tc.tile_pool(name="y", bufs=2))
            for lc in range(2):
                pp = ppsum.tile([128, 1024], f32)
                for hp in range(6):
                    lhs = xT[:, hp * LCP + lc * 128 : hp * LCP + (lc + 1) * 128]
                    nc.tensor.matmul(
                        pp[:, 0:512],
                        lhs,
                        pw[:, hp * DIM : hp * DIM + 512],
                        start=(hp == 0),
                        stop=(hp == 5),
                    )
                    nc.tensor.matmul(
                        pp[:, 512:768],
                        lhs,
                        pw[:, hp * DIM + 512 : hp * DIM + DIM],
                        start=(hp == 0),
                        stop=(hp == 5),
                    )
                yt = ypool.tile([128, DIM], f32)
                nc.vector.tensor_add(yt[:], pp[:, 0:DIM], pb[:])
                nc.scalar.dma_start(out_d[lc * 128 : (lc + 1) * 128, :], yt[:])

    nc.compile()
    return nc


def _host_prep(s0, s1, s2, v, fuse_w, fuse_b, proj_w, proj_b):
    """Build per-core input maps (bf16 packing; all transposes host-side)."""
    import ml_dtypes

    bf16 = ml_dtypes.bfloat16
    fuse_w = np.asarray(fuse_w, dtype=np.float32)
    fuse_b = np.asarray(fuse_b, dtype=np.float32)
    proj_w = np.asarray(proj_w, dtype=np.float32)
    proj_b = np.asarray(proj_b, dtype=np.float32)

    # scores quantized to int8 on a fixed +-6.0 grid (abs err ~0.024 LSB,
    # uniform; N(0,1) data never clips); the 6/127 scale is folded into wts
    SSCALE = 6.0 / 127.0
    sb = [
        np.clip(np.rint(np.asarray(s, dtype=np.float32) * (1.0 / SSCALE)), -127, 127)
        .astype(np.int8)
        for s in (s0, s1, s2)
    ]  # [B,12,L,T] int8
    vb = np.asarray(v, dtype=bf16)  # [B,12,T,D]

    # block-diag conv weights: w_j[k=(lg,c), m=(o,lg)] = fuse_w[o, 12j+c]*scale
    wts = np.zeros((3, KM, KM), dtype=bf16)
    for j in range(3):
        blk = (fuse_w[:, 12 * j : 12 * (j + 1)].T * SSCALE).astype(bf16)  # [c, o]
        for lg in range(G):
            wts[j, lg * 12 : (lg + 1) * 12, lg::G] = blk
    b120 = np.repeat(fuse_b, G).astype(np.float32).reshape(KM, 1)  # p = o*G+lg
    identb = np.eye(KM, dtype=bf16)
    # pwp[p, hp*768+o] = proj_w[o, hp*128+p]
    pwp = np.ascontiguousarray(
        proj_w.T.reshape(6, 128, DIM).transpose(1, 0, 2).reshape(128, 6 * DIM)
    ).astype(bf16)
    pbb = np.broadcast_to(proj_b, (128, DIM)).copy()

    in_maps = []
    for k in range(NCORES):
        b = k // (NCORES // B)
        l0 = (k % (NCORES // B)) * LC
        # sc[g, lg*12+c, j*T+t] = s_j[b, c, l0+g*10+lg, t]  (l padded 256->260)
        core = np.stack([s[b, :, l0 : l0 + LC, :] for s in sb])  # [3,12,256,T]
        pad = np.zeros((3, 12, LCP - LC, T), dtype=np.int8)
        corep = np.concatenate([core, pad], axis=2)  # [3,12,260,T]
        sc = np.ascontiguousarray(
            corep.reshape(3, 12, NG, G, T)
            .transpose(2, 3, 1, 0, 4)
            .reshape(NG, KM, 3 * T)
        )
        # vp[h, p, tc*64+d] = v[b, h, tc*128+p, d]
        vp = np.ascontiguousarray(
            vb[b].reshape(H, NTC, 128, D).transpose(0, 2, 1, 3).reshape(H, 128, NTC * D)
        )
        m = {
            "sc": sc,
            "vp": vp,
            "wts": wts,
            "b120": b120,
            "identb": identb,
            "pwp": pwp,
            "pbb": pbb,
        }
        in_maps.append(m)
    return in_maps


def _install_ntff_hook():
    """Provide antenv.axon_hooks (absent in this image) so trace=True works."""
    import os

    try:
        from antenv import axon_hooks  # noqa: F401

        return True
    except ImportError:
        pass
    try:
        import types
        import ctypes
        import contextlib
        import antenv

        so_path = "/opt/axon/libaxon_pjrt.so"
        if not os.path.exists(so_path):
            return False
        lib = ctypes.CDLL(so_path)
        if not hasattr(lib, "axon_start_nrt_profile"):
            return False
        lib.axon_start_nrt_profile.argtypes = [
            ctypes.POINTER(ctypes.c_int64),
            ctypes.c_size_t,
        ]
        lib.axon_start_nrt_profile.restype = ctypes.c_int64
        lib.axon_stop_nrt_profile.argtypes = [ctypes.c_char_p]
        lib.axon_stop_nrt_profile.restype = ctypes.c_int64

        @contextlib.contextmanager
        def _hook(output_dir, device_ids):
            import jax

            jax.devices()
            if device_ids:
                ids = (ctypes.c_int64 * len(device_ids))(*device_ids)
                rc = lib.axon_start_nrt_profile(ids, len(device_ids))
            else:
                rc = lib.axon_start_nrt_profile(None, 0)
            if rc != 0:
                raise RuntimeError(f"axon_start_nrt_profile rc={rc}")
            try:
                yield
            finally:
                n = lib.axon_stop_nrt_profile(str(output_dir).encode())
                print(f"ntff profile: {n} file(s) -> {output_dir}", file=sys.stderr)

        mod = types.ModuleType("antenv.axon_hooks")
        _h = {"hook": _hook}
        mod.set_axon_ntff_profile_hook = lambda h: _h.__setitem__("hook", h)
        mod.get_axon_ntff_profile_hook = lambda: _h["hook"]
        sys.modules["antenv.axon_hooks"] = mod
        antenv.axon_hooks = mod
        return True
    except Exception as e:  # degrade to untraced
        print("ntff hook install failed:", e, file=sys.stderr)
        return False


def kernel(s0, s1, s2, v, fuse_w, fuse_b, proj_w, proj_b, _trace=False):
    from concourse import bass_utils
    from concourse.bass_utils import run_bass_kernel_spmd

    if "nc" not in _CACHE:
        _CACHE["nc"] = _build_nc()
    nc = _CACHE["nc"]

    in_maps = _host_prep(s0, s1, s2, v, fuse_w, fuse_b, proj_w, proj_b)
    if _trace:
        _trace = _install_ntff_hook()
        bass_utils.upload_artifacts = lambda tmpdir: f"local:{tmpdir}"
    tmpdir = None
    if _trace:
        import tempfile

        tmpdir = tempfile.mkdtemp(prefix="bass_trace_")
        _CACHE["trace_dir"] = tmpdir
    try:
        res = run_bass_kernel_spmd(
            nc, in_maps, core_ids=list(range(NCORES)), trace=_trace, tmpdir=tmpdir
        )
    except Exception:
        if not _trace:
            raise
        import traceback

        traceback.print_exc()
        print("trace run failed; retrying untraced", file=sys.stderr)
        res = run_bass_kernel_spmd(nc, in_maps, core_ids=list(range(NCORES)))
    _CACHE["last_exec_time_ns"] = res.exec_time_ns
    _CACHE["last_results"] = res

    out = np.empty((B, L, DIM), dtype=np.float32)
    for k in range(NCORES):
        b = k // (NCORES // B)
        l0 = (k % (NCORES // B)) * LC
        out[b, l0 : l0 + LC, :] = res.results[k]["out"]
    return out


# revision 31
# speedup vs baseline: 1.1738x; 1.1738x over previous
"""Fused conv-attention kernel for Trainium2, sharded over 8 NeuronCores.

Reference computation (B=2, H=12, L=T=1024, D=64, FEA=3, DIM=768):
    scores = concat([s0,s1,s2], ch)            # [b, 36, l, t]
    fused  = einsum('bclt,oc->bolt', scores, fuse_w) + fuse_b
    attn   = softmax(fused, axis=-1)
    x      = einsum('bhlt,bhtd->bhld', attn, v)
    y      = merge_heads(x) @ proj_w.T + proj_b  # [b, l, 768]

Sharding: fully data-parallel over (b, l-block): core k handles b=k//4 and
l-rows [256*(k%4), 256*(k%4)+256).  Every op is local; no collectives.

All HBM traffic is bf16 (host casts/packs): scores 19.2MB/core with 2KB
descriptors so the DMA stripes across all 16 engines; v/proj weights flow on
the second HWDGE queue (scalar engine) in parallel with the score stream.

Per-core dataflow (G=10 l-rows per group, KM=120 partitions, LCP=260 padded):
  - conv as block-diag matmul: lhsT_j [120,120] holds fuse_w cols for score
    tensor j replicated block-diagonally over lg; j-outer loop -> 3 stationary
    loads per group, 6 matmuls of N=512 into PSUM [120,1024].
  - exp via ScalarE activation (bias=fuse_b, accum_out=row sums z); max
    subtraction skipped (|fused| <= ~5 so fp32 exp is safe).
  - softmax normalize FUSED into the PE transpose: stationary = et t-chunk,
    moving = diag(1/z) bf16 -> psum tp[t,120] holds normalized attn^T.
  - attnT sbuf layout [t-part, (tc, g, o, lg)] bf16: psum->sbuf copies are
    contiguous; phase-2 matmul rhs reads the strided (g,lg) view (N=260).
  - attn @ V: per (h, tc) matmul accumulating x^T [64, 260] over 8 t-chunks.
  - final proj: out[l,768] = sum_hp xT_hp^T @ pw_hp, bias added by DVE.
"""

import sys

import numpy as np

sys.path.insert(0, "/opt/trn_rl_repo")

B, H, L, T, D = 2, 12, 1024, 1024, 64
DIM = H * D  # 768
NCORES = 8
LC = L * B // NCORES  # 256 l-rows per core
G = 10  # l-rows per conv group
NG = 26  # groups per core (last is padded: 260 > 256)
LCP = NG * G  # 260
KM = 12 * G  # 120: conv matmul K and M
NTC = T // 128  # 8 t-chunks

_CACHE = {}


def _build_nc():
    import concourse.bacc as bacc
    import concourse.mybir as mybir
    import concourse.tile as tile
    from contextlib import ExitStack

    f32 = mybir.dt.float32
    bf16 = mybir.dt.bfloat16

    nc = bacc.Bacc(
        "TRN2", target_bir_lowering=False, debug=False, enable_asserts=False
    )

    sc_in = nc.dram_tensor("sc", [NG, KM, 3 * T], bf16, kind="ExternalInput").ap()
    v_in = nc.dram_tensor("vp", [H, 128, NTC * D], bf16, kind="ExternalInput").ap()
    w_in = nc.dram_tensor("wts", [3, KM, KM], bf16, kind="ExternalInput").ap()
    b_in = nc.dram_tensor("b120", [KM, 1], f32, kind="ExternalInput").ap()
    id_in = nc.dram_tensor("identb", [KM, KM], bf16, kind="ExternalInput").ap()
    pw_in = nc.dram_tensor("pwp", [128, 6 * DIM], bf16, kind="ExternalInput").ap()
    pb_in = nc.dram_tensor("pbb", [128, DIM], f32, kind="ExternalInput").ap()
    out_d = nc.dram_tensor("out", [LC, DIM], f32, kind="ExternalOutput").ap()

    with tile.TileContext(nc) as tc, ExitStack() as ctx:
        # ---- persistent SBUF (weights arrive on the scalar HWDGE queue) ----
        singles = ctx.enter_context(tc.tile_pool(name="singles", bufs=1))
        wt = [singles.tile([KM, KM], bf16, tag=f"wt{j}", name=f"wt{j}") for j in range(3)]
        for j in range(3):
            nc.scalar.dma_start(wt[j][:], w_in[j])
        b120 = singles.tile([KM, 1], f32)
        nc.scalar.dma_start(b120[:], b_in)
        identb = singles.tile([KM, KM], bf16)
        nc.scalar.dma_start(identb[:], id_in)
        vsb = singles.tile([128, H * NTC * D], bf16)  # [t%128, h*512 + tc*64 + d]
        for h in range(H):
            nc.scalar.dma_start(vsb[:, h * 512 : (h + 1) * 512], v_in[h])
        pw = singles.tile([128, 6 * DIM], bf16)  # [i%128, hp*768 + o]
        for q in range(3):
            nc.scalar.dma_start(
                pw[:, q * 1536 : (q + 1) * 1536], pw_in[:, q * 1536 : (q + 1) * 1536]
            )
        pb = singles.tile([128, DIM], f32)
        nc.scalar.dma_start(pb[:], pb_in)
        # attn^T accumulator: [t%128, tc*3120 + o*260 + (g*10+lg)]  bf16
        attnT = singles.tile([128, NTC * NG * KM], bf16)
        # x^T for proj: [i%128 part, (i//128)*260 + l]  bf16
        xT = singles.tile([128, 6 * LCP], bf16)

        # ---- phase 1: conv + softmax + normalized transpose, per group ----
        with ExitStack() as p1:
            spool = p1.enter_context(tc.tile_pool(name="scores", bufs=4))
            fpsum = p1.enter_context(
                tc.tile_pool(name="fpsum", bufs=2, space="PSUM")
            )
            epool = p1.enter_context(tc.tile_pool(name="exp", bufs=3))
            zpool = p1.enter_context(tc.tile_pool(name="z", bufs=4))
            dpool = p1.enter_context(tc.tile_pool(name="diag", bufs=2))
            tpsum = p1.enter_context(
                tc.tile_pool(name="tpsum", bufs=4, space="PSUM")
            )
            for g in range(NG):
                st_t = spool.tile([KM, 3 * T], bf16, tag="st", name=f"st_{g}")
                # alternate HWDGE queues (sync/scalar) to double queue throughput
                qeng = nc.sync if g % 2 == 0 else nc.scalar
                qeng.dma_start(st_t[:], sc_in[g])
                st = st_t[:]
                fp = fpsum.tile([KM, T], f32)
                for j in range(3):
                    for th in range(2):
                        nc.tensor.matmul(
                            fp[:, th * 512 : (th + 1) * 512],
                            wt[j][:],
                            st[:, j * T + th * 512 : j * T + (th + 1) * 512],
                            start=(j == 0),
                            stop=(j == 2),
                        )
                et = epool.tile([KM, T], bf16, tag="et")
                zt = zpool.tile([KM, 1], f32, tag="zt")
                nc.scalar.activation(
                    et[:],
                    fp[:],
                    mybir.ActivationFunctionType.Exp,
                    bias=b120[:],
                    accum_out=zt[:],
                )
                zi = zpool.tile([KM, 1], f32, tag="zi")
                nc.vector.reciprocal(zi[:], zt[:])
                dg = dpool.tile([KM, KM], bf16, tag="dg")
                nc.vector.tensor_scalar_mul(dg[:], identb[:], zi[:])
                for half in range(2):
                    tp = tpsum.tile([128, 4 * KM], f32)
                    for q in range(4):
                        tc_i = half * 4 + q
                        nc.tensor.matmul(
                            tp[:, q * KM : (q + 1) * KM],
                            et[:, tc_i * 128 : (tc_i + 1) * 128],
                            dg[:],
                            start=True,
                            stop=True,
                        )
                    dst = (
                        attnT[:]
                        .rearrange("p (tc o l) -> p tc o l", tc=NTC, o=H)[
                            :, half * 4 : (half + 1) * 4, :, g * G : (g + 1) * G
                        ]
                    )
                    nc.vector.tensor_copy(
                        dst, tp[:].rearrange("p (q o lg) -> p q o lg", q=4, o=H)
                    )

        # ---- phase 2: attn @ V  -> x^T ----
        with ExitStack() as p2:
            xpsum = p2.enter_context(
                tc.tile_pool(name="xpsum", bufs=4, space="PSUM")
            )
            for h in range(H):
                xp = xpsum.tile([D, LCP], f32)
                for tc_i in range(NTC):
                    nc.tensor.matmul(
                        xp[:],
                        vsb[:, h * 512 + tc_i * D : h * 512 + (tc_i + 1) * D],
                        attnT[
                            :,
                            tc_i * H * LCP + h * LCP : tc_i * H * LCP + (h + 1) * LCP,
                        ],
                        start=(tc_i == 0),
                        stop=(tc_i == NTC - 1),
                    )
                po = (h % 2) * D
                ko = (h // 2) * LCP
                nc.vector.tensor_copy(xT[po : po + D, ko : ko + LCP], xp[:])

            # ---- phase 3: proj -> out ----
            ppsum = p2.enter_context(
                tc.tile_pool(name="ppsum", bufs=2, space="PSUM")
            )
            ypool = p2.enter_context(tc.tile_pool(name="y", bufs=2))
            for lc in range(2):
                pp = ppsum.tile([128, 1024], f32)
                for hp in range(6):
                    lhs = xT[:, hp * LCP + lc * 128 : hp * LCP + (lc + 1) * 128]
                    nc.tensor.matmul(
                        pp[:, 0:512],
                        lhs,
                        pw[:, hp * DIM : hp * DIM + 512],
                        start=(hp == 0),
                        stop=(hp == 5),
                    )
                    nc.tensor.matmul(
                        pp[:, 512:768],
                        lhs,
                        pw[:, hp * DIM + 512 : hp * DIM + DIM],
                        start=(hp == 0),
                        stop=(hp == 5),
                    )
                yt = ypool.tile([128, DIM], f32)
                nc.vector.tensor_add(yt[:], pp[:, 0:DIM], pb[:])
                nc.scalar.dma_start(out_d[lc * 128 : (lc + 1) * 128, :], yt[:])

    nc.compile()
    return nc


def _host_prep(s0, s1, s2, v, fuse_w, fuse_b, proj_w, proj_b):
    """Build per-core input maps (bf16 packing; all transposes host-side)."""
    import ml_dtypes

    bf16 = ml_dtypes.bfloat16
    fuse_w = np.asarray(fuse_w, dtype=np.float32)
    fuse_b = np.asarray(fuse_b, dtype=np.float32)
    proj_w = np.asarray(proj_w, dtype=np.float32)
    proj_b = np.asarray(proj_b, dtype=np.float32)

    sb = [np.asarray(s, dtype=bf16) for s in (s0, s1, s2)]  # [B,12,L,T] bf16
    vb = np.asarray(v, dtype=bf16)  # [B,12,T,D]

    # block-diag conv weights: w_j[k=(lg,c), m=(o,lg)] = fuse_w[o, 12j+c]
    wts = np.zeros((3, KM, KM), dtype=bf16)
    for j in range(3):
        blk = fuse_w[:, 12 * j : 12 * (j + 1)].T.astype(bf16)  # [c, o]
        for lg in range(G):
            wts[j, lg * 12 : (lg + 1) * 12, lg::G] = blk
    b120 = np.repeat(fuse_b, G).astype(np.float32).reshape(KM, 1)  # p = o*G+lg
    identb = np.eye(KM, dtype=bf16)
    # pwp[p, hp*768+o] = proj_w[o, hp*128+p]
    pwp = np.ascontiguousarray(
        proj_w.T.reshape(6, 128, DIM).transpose(1, 0, 2).reshape(128, 6 * DIM)
    ).astype(bf16)
    pbb = np.broadcast_to(proj_b, (128, DIM)).copy()

    in_maps = []
    for k in range(NCORES):
        b = k // (NCORES // B)
        l0 = (k % (NCORES // B)) * LC
        # sc[g, lg*12+c, j*T+t] = s_j[b, c, l0+g*10+lg, t]  (l padded 256->260)
        core = np.stack([s[b, :, l0 : l0 + LC, :] for s in sb])  # [3,12,256,T]
        pad = np.zeros((3, 12, LCP - LC, T), dtype=bf16)
        corep = np.concatenate([core, pad], axis=2)  # [3,12,260,T]
        sc = np.ascontiguousarray(
            corep.reshape(3, 12, NG, G, T)
            .transpose(2, 3, 1, 0, 4)
            .reshape(NG, KM, 3 * T)
        )
        # vp[h, p, tc*64+d] = v[b, h, tc*128+p, d]
        vp = np.ascontiguousarray(
            vb[b].reshape(H, NTC, 128, D).transpose(0, 2, 1, 3).reshape(H, 128, NTC * D)
        )
        m = {
            "sc": sc,
            "vp": vp,
            "wts": wts,
            "b120": b120,
            "identb": identb,
            "pwp": pwp,
            "pbb": pbb,
        }
        in_maps.append(m)
    return in_maps


def _install_ntff_hook():
    """Provide antenv.axon_hooks (absent in this image) so trace=True works."""
    import os

    try:
        from antenv import axon_hooks  # noqa: F401

        return True
    except ImportError:
        pass
    try:
        import types
        import ctypes
        import contextlib
        import antenv

        so_path = "/opt/axon/libaxon_pjrt.so"
        if not os.path.exists(so_path):
            return False
        lib = ctypes.CDLL(so_path)
        if not hasattr(lib, "axon_start_nrt_profile"):
            return False
        lib.axon_start_nrt_profile.argtypes = [
            ctypes.POINTER(ctypes.c_int64),
            ctypes.c_size_t,
        ]
        lib.axon_start_nrt_profile.restype = ctypes.c_int64
        lib.axon_stop_nrt_profile.argtypes = [ctypes.c_char_p]
        lib.axon_stop_nrt_profile.restype = ctypes.c_int64

        @contextlib.contextmanager
        def _hook(output_dir, device_ids):
            import jax

            jax.devices()
            if device_ids:
                ids = (ctypes.c_int64 * len(device_ids))(*device_ids)
                rc = lib.axon_start_nrt_profile(ids, len(device_ids))
            else:
                rc = lib.axon_start_nrt_profile(None, 0)
            if rc != 0:
                raise RuntimeError(f"axon_start_nrt_profile rc={rc}")
            try:
                yield
            finally:
                n = lib.axon_stop_nrt_profile(str(output_dir).encode())
                print(f"ntff profile: {n} file(s) -> {output_dir}", file=sys.stderr)

        mod = types.ModuleType("antenv.axon_hooks")
        _h = {"hook": _hook}
        mod.set_axon_ntff_profile_hook = lambda h: _h.__setitem__("hook", h)
        mod.get_axon_ntff_profile_hook = lambda: _h["hook"]
        sys.modules["antenv.axon_hooks"] = mod
        antenv.axon_hooks = mod
        return True
    except Exception as e:  # degrade to untraced
        print("ntff hook install failed:", e, file=sys.stderr)
        return False


def kernel(s0, s1, s2, v, fuse_w, fuse_b, proj_w, proj_b, _trace=False):
    from concourse import bass_utils
    from concourse.bass_utils import run_bass_kernel_spmd

    if "nc" not in _CACHE:
        _CACHE["nc"] = _build_nc()
    nc = _CACHE["nc"]

    in_maps = _host_prep(s0, s1, s2, v, fuse_w, fuse_b, proj_w, proj_b)
    if _trace:
        _trace = _install_ntff_hook()
        bass_utils.upload_artifacts = lambda tmpdir: f"local:{tmpdir}"
    tmpdir = None
    if _trace:
        import tempfile

        tmpdir = tempfile.mkdtemp(prefix="bass_trace_")
        _CACHE["trace_dir"] = tmpdir
    try:
        res = run_bass_kernel_spmd(
            nc, in_maps, core_ids=list(range(NCORES)), trace=_trace, tmpdir=tmpdir
        )
    except Exception:
        if not _trace:
            raise
        import traceback

        traceback.print_exc()
        print("trace run failed; retrying untraced", file=sys.stderr)
        res = run_bass_kernel_spmd(nc, in_maps, core_ids=list(range(NCORES)))
    _CACHE["last_exec_time_ns"] = res.exec_time_ns
    _CACHE["last_results"] = res

    out = np.empty((B, L, DIM), dtype=np.float32)
    for k in range(NCORES):
        b = k // (NCORES // B)
        l0 = (k % (NCORES // B)) * LC
        out[b, l0 : l0 + LC, :] = res.results[k]["out"]
    return out
